# revision 28
# baseline (speedup 1.0000x reference)
"""NestedAttention Trainium2 kernel (v6).

Reference computation (per batch b):
  q_i = wq[i] @ x ; k_j = wk[j] @ x ; v_j = wv[j] @ x        (1x1 convs, r=64)
  for i: acc_i = sum_j softmax_m(q_i^T k_j / sqrt(r)) applied to v_j
  out = wo @ concat_i(acc_i) ; y = x * sigmoid(out)

Sharding: 8 cores = batch(4) x query-column-halves(2). Each core holds full
k/v (m = 2304 keys) and a 1152-wide slice of query columns n.

v6 final (196us HW, rel err 6.5e-3; v2 baseline was 219.5us):
  - 3-deep mm1 PSUM pipeline: main slots are [128,1024] f32 (2 banks) with
    bufs=3, so mm1 m-tile t+3 never waits on exp(t) (the v2 structure's
    serial exp->mm1 chain cost ~160ns isolated-MM refill per m-tile plus
    stalls).  The 128-col m-tile tails collect 4-up into shared mm2-pool
    [128,512] slots and get one batched strided-AP exp per group.
  - mm2/norm chunks emit at staggered cuts (4,9,14) of the next pair's mm1
    stream so tail-collector and mm2 allocations alternate cleanly in the
    2-slot mm2 pool rotation.
  - Host tensors partition-major ([P, KT, ...]) -> contiguous input DMA;
    x8 (fp8) loads before xb (bf16, only needed at the end); log2(e)
    prescale folded into wq on the host; wq/wk zero-padded to 128 output
    cols (NOT duplicated - K=128 contraction would double the logits).
  - PE warm-up burst during the DMA phase lifts the HAM clock gate before
    the first real matmul; first MM ~7us (was 15.4us).
  - ScalarE exp uses bias -1 (whole-column e^-1 scale, softmax-invariant
    via the ones-rows Z) so extreme logits can't hit fp8e4 Inf codes;
    VectorE Schraudolph writes through a uint8 view so negative bit values
    saturate to 0 (= correct exp underflow) instead of becoming 0xFF NaN.
    (v2 relied on luck: unclamped ScalarE m-tiles / int8 negatives.)
  - y stored bf16 (host casts to f32); final matmul contracts K=64 (no
    zero padding); vT ones via one contiguous memset.
  - Row-tiled mm1 (tile_position (0,0)/(64,0), K=64) was tried in two
    emission orders and NEVER ran concurrently on HW (serial abutment in
    traces) while adding 64<->128 tiling-mode drain penalties - abandoned.

Kept from v2: fp8 E + DoubleRow mm2 with in-matmul ones-columns for the
softmax normalizer; exp split between ScalarE (ACT Exp -> fp8) and VectorE
(Schraudolph bit-trick: bits_e4m3(exp(S/8)) = trunc(log2e*S + 56.5-C),
one tensor_scalar add+min into an int8 view); reciprocal via one 64-row
shifted copy + reciprocal_approx_fast (plain copies may partition-shift,
reciprocal may not); GPSIMD j-accumulation adds; sigmoid as
0.5*(1+tanh(z/2)) so exp+tanh share one ACT table set.
"""

import os
import numpy as np

B, C, H, W = 4, 256, 48, 48
N = H * W            # 2304 keys (m) per image
NSLICE = N // 2      # 1152 query columns (n) per core
R = 64               # reduced channels
P = 128
MT = N // P          # 18 m-tiles
NG = MT // 2         # 9 double-row groups of 256 keys
KT = C // P          # 2 contraction tiles over channels
CHUNKS = [(0, 512), (512, 512), (1024, 128)]  # n chunks, PSUM-bank aligned
N_CORES = 8

LOG2E = float(np.log2(np.e))
# q is prescaled (on host, folded into wq) by log2e so mm1 logits are
# Sb = log2e * S.
#   VectorE path: e4m3 bits = Sb + 56.5 - C  (trunc int8 convert)
#   ScalarE path: exp(S/8) = exp(Sb * ln2/8) -> scale = ln(2)/8
ACT_SCALE = float(np.log(2.0) / 8.0)
BITS_C = float(os.environ.get("NESTED_BITS_C", "0.46"))
BITS_BIAS = 56.5 - BITS_C      # trunc() semantics of the int8 convert
BITS_CLAMP = 119.0             # max e4m3 bits (=240.0); >=120 is Inf/NaN on TRN

SSPLIT = int(os.environ.get("NESTED_SSPLIT", "11"))  # of 18 m-tiles on ScalarE
MID = int(os.environ.get("NESTED_MID", "8"))         # mm2/norm emission point
ADDS_ENGINE = os.environ.get("NESTED_ADDS", "dve")    # gpsimd | dve
RECIP_MODE = os.environ.get("NESTED_RECIP", "copy1")
MM1_TILED = os.environ.get("NESTED_MM1_TILED", "0") == "1"
# Column split of each m-tile's exp between ScalarE [0:SCOL] and VectorE
# [SCOL:NSLICE].  Both engines work on every m-tile concurrently, so the
# PSUM slot frees ~2x earlier than whole-tile engine alternation, removing
# the exp->mm1 refill stalls (subtile deps release each chunk as its
# engine's piece finishes).  0 disables (whole-tile SSPLIT assignment).
# Measured: the per-instruction overhead (~231ns ScalarE / ~137ns DVE)
# doubles under the split and eats the stall savings -> default off.
SCOL = int(os.environ.get("NESTED_SCOL", "0"))
# v6 pipeline: main psum slots are [128, 1024] (2 banks) with bufs=3, so
# mm1 runs a 3-deep rotation against the exps (no serial exp->mm1 chain).
# The 128-col m-tile tails collect 4-up into shared mm2-pool slots and get
# one batched exp per group of 4.
MAIN = 1024
TAIL = NSLICE - MAIN           # 128
TAIL_GROUPS = [(0, 4), (4, 4), (8, 4), (12, 4), (16, 2)]
MCHUNKS = [(0, 512), (512, 512)]
BIG_BUFS = int(os.environ.get("NESTED_BIG_BUFS", "3"))
# mm2/norm chunk emission points inside the next pair's mm1 stream --
# staggered so tail-collector and mm2 allocations alternate cleanly in the
# 2-slot mm2 pool rotation.
CUTS = [int(x) for x in os.environ.get("NESTED_CUTS", "4,9,14").split(",")]
# tail-exp groups on the DVE bit-trick path (rest on ScalarE)
TAIL_DVE = set(
    int(x) for x in os.environ.get("NESTED_TAIL_DVE", "0,4").split(",") if x != ""
)
FINAL_GPSIMD = os.environ.get("NESTED_FINAL_GPSIMD", "0") == "1"
WARM_MMS = int(os.environ.get("NESTED_WARM_MMS", "70"))

_CACHE = {}
LAST_RESULTS = None


def _dve_mts(ssplit):
    """Which m-tiles go to the VectorE Schraudolph path (spread evenly).
    Phase-shifted so mt0 stays on ScalarE (VectorE often still drains the
    previous pair's normalization at pair start)."""
    d = MT - ssplit
    ph = int(os.environ.get("NESTED_DVE_PHASE", "0"))
    return set(mt for mt in range(MT) if ((mt + ph) * d) % MT < d)


def _build_program():
    from contextlib import ExitStack

    import concourse.bass as bass
    import concourse.tile as tile
    from concourse import bacc, mybir

    f32 = mybir.dt.float32
    bf16 = mybir.dt.bfloat16
    fp8 = mybir.dt.float8e4
    i8 = mybir.dt.int8
    u8 = mybir.dt.uint8
    Exp = mybir.ActivationFunctionType.Exp
    Tanh = mybir.ActivationFunctionType.Tanh
    DoubleRow = mybir.MatmulPerfMode.DoubleRow
    mult = mybir.AluOpType.mult
    add = mybir.AluOpType.add
    min_op = mybir.AluOpType.min

    nc = bacc.Bacc("TRN2", target_bir_lowering=False, debug=False)
    # All DRAM tensors are partition-major so input DMA is contiguous.
    # xb is column-rotated per core so the core's own n-slice is columns
    # 0:NSLICE (softmax/mm2 are permutation-invariant over keys m).
    x8_d = nc.declare_dram_parameter("x8", [P, KT, N], fp8, isOutput=False)
    xb_d = nc.declare_dram_parameter("xb", [P, KT, N], bf16, isOutput=False)
    # wq/wk duplicated along output dim (2R = 128) for row-tiled mm1.
    wqT_d = nc.declare_dram_parameter("wqT", [P, KT, 3, P], fp8, isOutput=False)
    wkT_d = nc.declare_dram_parameter("wkT", [P, KT, 3, P], fp8, isOutput=False)
    wvT_d = nc.declare_dram_parameter("wvT", [P, KT, 3, R], fp8, isOutput=False)
    woT_d = nc.declare_dram_parameter("woT", [3, R, C], bf16, isOutput=False)
    y_d = nc.declare_dram_parameter("y", [KT, P, NSLICE], bf16, isOutput=True)

    with tile.TileContext(nc) as tc, ExitStack() as ctx:
        consts = ctx.enter_context(tc.tile_pool(name="consts", bufs=1))
        big_ps = ctx.enter_context(tc.tile_pool(name="big_ps", bufs=BIG_BUFS, space="PSUM"))
        mm2_ps = ctx.enter_context(tc.tile_pool(name="mm2_ps", bufs=2, space="PSUM"))
        e_pool = ctx.enter_context(tc.tile_pool(name="e_pool", bufs=3))
        rb_pool = ctx.enter_context(tc.tile_pool(name="rb_pool", bufs=3))
        small = ctx.enter_context(tc.tile_pool(name="small", bufs=3))

        # ---- PE warm-up: lift the HAM clock gate during the DMA phase ----
        # (targets a big_ps slot; the pool recycles it before the first
        # projection needs a third allocation)
        warm = consts.tile([P, 8], f32, tag="warm")
        nc.vector.memset(warm[:], 1.0)
        warm_ps = big_ps.tile([P, MAIN], f32, tag="big", name="warm_ps")
        for _ in range(WARM_MMS):
            nc.tensor.matmul(warm_ps[0:8, 0:8], warm[:, 0:8], warm[:, 0:8],
                             start=True, stop=True)

        # ---- persistent SBUF state ----
        wqT_sb = consts.tile([P, KT, 3, P], fp8)
        nc.sync.dma_start(wqT_sb[:], wqT_d[:])
        wkT_sb = consts.tile([P, KT, 3, P], fp8)
        nc.sync.dma_start(wkT_sb[:], wkT_d[:])
        wvT_sb = consts.tile([P, KT, 3, R], fp8)
        nc.sync.dma_start(wvT_sb[:], wvT_d[:])
        x8_sb = consts.tile([P, KT, N], fp8)
        nc.sync.dma_start(x8_sb[:, :, 0:NSLICE], x8_d[:, :, 0:NSLICE])
        nc.sync.dma_start(x8_sb[:, :, NSLICE:N], x8_d[:, :, NSLICE:N])
        x_sb = consts.tile([P, KT, N], bf16)
        nc.sync.dma_start(x_sb[:, :, 0:NSLICE], xb_d[:, :, 0:NSLICE])
        nc.sync.dma_start(x_sb[:, :, NSLICE:N], xb_d[:, :, NSLICE:N])

        woT_sb = []
        for i in range(3):
            w = consts.tile([R, C], bf16, tag=f"woT{i}")
            nc.sync.dma_start(w[:], woT_d[i])
            woT_sb.append(w)

        q_sb = consts.tile([P, 3, NSLICE], bf16)
        k_sb = consts.tile([P, 3, N], bf16)

        # vT buffer per 256-group: [g, t, j, 0:64]=v_j fp8, [g, t, j, 64:128]=1
        # One contiguous memset; the v cast overwrites [0:64].
        vT_buf = consts.tile([P, NG, 2, 3, P], fp8)
        nc.gpsimd.memset(vT_buf[:], 1.0)

        # acc_i accumulated in bf16 on 64 partitions (final mm contracts K=64)
        acc = []
        for i in range(3):
            a = consts.tile([R, NSLICE], bf16, tag=f"acc{i}")
            acc.append(a)

        # per-partition bias vector for the ScalarE exp (see emit_exp)
        ebias = consts.tile([P, 1], f32, tag="ebias")
        nc.vector.memset(ebias[:], -1.0)

        # warm the ACT exp table + GPSIMD tensor kernels during the DMA phase
        nc.scalar.activation(warm[:, 0:4], warm[:, 4:8], Exp, scale=0.1)
        nc.gpsimd.tensor_tensor(warm[:, 0:4], warm[:, 4:8], warm[:, 4:8], add)
        nc.gpsimd.tensor_scalar(warm[:, 0:4], warm[:, 4:8], 1.0, None, add)

        # ---- projections ----
        def emit_proj(wT_sb, jj, dst, base):
            pt = big_ps.tile([P, MAIN], f32, tag="big", name="pt_main")
            pt2 = big_ps.tile([P, MAIN], f32, tag="big", name="pt_tail")
            for c0, cw in MCHUNKS:
                nc.tensor.matmul(
                    pt[:, c0 : c0 + cw],
                    wT_sb[:, :, jj, :],
                    x8_sb[:, :, base + c0 : base + c0 + cw],
                    start=True,
                    stop=True,
                    perf_mode=DoubleRow,
                )
            nc.tensor.matmul(
                pt2[:, 0:TAIL],
                wT_sb[:, :, jj, :],
                x8_sb[:, :, base + MAIN : base + NSLICE],
                start=True,
                stop=True,
                perf_mode=DoubleRow,
            )
            nc.scalar.copy(dst[:, 0:MAIN], pt[:])
            nc.scalar.copy(dst[:, MAIN:NSLICE], pt2[:, 0:TAIL])

        def emit_q(i):
            # log2e prescale is folded into wq on the host
            emit_proj(wqT_sb, i, q_sb[:, i, :], 0)

        def emit_k(j, halves=(0, 1)):
            for half in halves:
                emit_proj(
                    wkT_sb, j,
                    k_sb[:, j, half * NSLICE : (half + 1) * NSLICE],
                    half * NSLICE,
                )

        def emit_vT_all():
            for mt in range(MT):
                g, t = mt // 2, mt % 2
                pv = mm2_ps.tile([P, 512], f32, tag="mm2")
                nc.tensor.matmul(
                    pv[:, 0 : 3 * R],
                    x8_sb[:, :, mt * P : (mt + 1) * P],
                    wvT_sb[:, :, :, :],
                    start=True,
                    stop=True,
                    perf_mode=DoubleRow,
                )
                nc.vector.tensor_copy(
                    vT_buf[:, g, t, :, 0:R],
                    pv[:, 0 : 3 * R].rearrange("p (j r) -> p j r", j=3),
                )

        # ---- attention pair pipeline ----
        def emit_exp_main(E, mt, pt, dve_mts):
            if mt in dve_mts:
                nc.vector.tensor_scalar(
                    E[:, mt, 0:MAIN].bitcast(u8),
                    pt[:],
                    BITS_BIAS,
                    BITS_CLAMP,
                    add,
                    min_op,
                )
            else:
                # bias -1: exp(S/8 - 1) scales whole columns by e^-1
                # (softmax-invariant via the ones-rows Z) and moves the
                # fp8e4 Inf threshold out to ~7 sigma of the logits.
                nc.scalar.activation(
                    E[:, mt, 0:MAIN], pt[:], Exp, scale=ACT_SCALE,
                    bias=ebias[:],
                )

        def emit_exp_tail(E, gi, g0, gn, coll):
            out = E[:, g0 : g0 + gn, MAIN:NSLICE]
            src_ap = coll[:, 0 : gn * TAIL].rearrange("p (g t) -> p g t", g=gn)
            if gi in TAIL_DVE:
                nc.vector.tensor_scalar(
                    out.bitcast(u8),
                    src_ap,
                    BITS_BIAS,
                    BITS_CLAMP,
                    add,
                    min_op,
                )
            else:
                nc.scalar.activation(
                    out, src_ap, Exp, scale=ACT_SCALE,
                    bias=ebias[:],
                )

        tail_state = {}

        def emit_mm1_exp(i, j, E, mts, ssplit=SSPLIT):
            dve_mts = _dve_mts(ssplit)
            for mt in mts:
                gi = min(mt // 4, len(TAIL_GROUPS) - 1)
                g0, gn = TAIL_GROUPS[gi]
                if mt == g0:
                    tail_state["coll"] = mm2_ps.tile(
                        [P, 512], f32, tag="mm2", name="tailc"
                    )
                pt = big_ps.tile([P, MAIN], f32, tag="big")
                for c0, cw in MCHUNKS:
                    nc.tensor.matmul(
                        pt[:, c0 : c0 + cw],
                        k_sb[:, j, mt * P : (mt + 1) * P],
                        q_sb[:, i, c0 : c0 + cw],
                        start=True,
                        stop=True,
                    )
                coll = tail_state["coll"]
                nc.tensor.matmul(
                    coll[:, (mt - g0) * TAIL : (mt - g0 + 1) * TAIL],
                    k_sb[:, j, mt * P : (mt + 1) * P],
                    q_sb[:, i, MAIN:NSLICE],
                    start=True,
                    stop=True,
                )
                emit_exp_main(E, mt, pt, dve_mts)
                if mt == g0 + gn - 1:
                    emit_exp_tail(E, gi, g0, gn, coll)

        def emit_final_chunk(po, c0, cw):
            def tgt(ct):
                if c0 < MAIN:
                    return po[ct][:, c0 : c0 + cw]
                return po[2][:, ct * TAIL : ct * TAIL + cw]
            for ct in range(KT):
                for i in range(3):
                    nc.tensor.matmul(
                        tgt(ct),
                        woT_sb[i][:, ct * P : (ct + 1) * P],
                        acc[i][:, c0 : c0 + cw],
                        start=(i == 0),
                        stop=(i == 2),
                    )
            for ct in range(KT):
                th = small.tile([P, 512], bf16, tag=f"th{ct}")
                # sigmoid(z) = 0.5*(1+tanh(z/2)); host folds the 0.5 into wo?
                # no - scale=0.5 here computes tanh(z/2).
                nc.scalar.activation(
                    th[:, 0:cw], tgt(ct), Tanh, scale=0.5
                )
                y_sb = small.tile([P, 512], bf16, tag=f"ysb{ct}")
                eng = nc.gpsimd if FINAL_GPSIMD else nc.vector
                eng.tensor_scalar(
                    th[:, 0:cw], th[:, 0:cw], 0.5, 0.5, mult, add
                )
                eng.tensor_tensor(
                    y_sb[:, 0:cw],
                    x_sb[:, ct, c0 : c0 + cw],
                    th[:, 0:cw],
                    mult,
                )
                deng = nc.sync if ct == 0 else nc.scalar
                deng.dma_start(y_d[ct][:, c0 : c0 + cw], y_sb[:, 0:cw])

        def emit_mm2_norm(i, j, E, po=None, chunks=None):
            for c0, cw in chunks if chunks is not None else CHUNKS:
                pa = mm2_ps.tile([P, 512], f32, tag="mm2")
                from contextlib import nullcontext
                # Under row tiling, mm2 matmuls are full-array (mode switch =
                # PE drain) so they run at low priority to only fill genuine
                # stall windows; untiled they interleave freely.
                prio = tc.high_priority(offset=-(1 << 20)) if MM1_TILED else nullcontext()
                with prio:
                    for g in range(NG):
                        nc.tensor.matmul(
                            pa[:, 0:cw],
                            vT_buf[:, g, :, j, :],
                            E[:, 2 * g : 2 * g + 2, c0 : c0 + cw],
                            start=(g == 0),
                            stop=(g == NG - 1),
                            perf_mode=DoubleRow,
                        )
                rb = rb_pool.tile([R, 512], f32, tag="rb")
                if RECIP_MODE == "copy1":
                    nc.vector.tensor_copy(rb[:, 0:cw], pa[R:P, 0:cw])
                    nc.vector.reciprocal_approx_fast(rb[:, 0:cw], rb[:, 0:cw])
                else:
                    nc.vector.tensor_copy(rb[0:32, 0:cw], pa[R : R + 32, 0:cw])
                    nc.vector.tensor_copy(rb[32:R, 0:cw], pa[R + 32 : P, 0:cw])
                    nc.vector.reciprocal_approx_fast(rb[:, 0:cw], rb[:, 0:cw])
                if j == 0:
                    nc.vector.tensor_tensor(
                        acc[i][:, c0 : c0 + cw], pa[0:R, 0:cw], rb[:, 0:cw], mult
                    )
                else:
                    tmp = small.tile([R, 512], bf16, tag="tmp")
                    nc.vector.tensor_tensor(
                        tmp[:, 0:cw], pa[0:R, 0:cw], rb[:, 0:cw], mult
                    )
                    eng = nc.gpsimd if ADDS_ENGINE == "gpsimd" else nc.vector
                    eng.tensor_tensor(
                        acc[i][:, c0 : c0 + cw],
                        acc[i][:, c0 : c0 + cw],
                        tmp[:, 0:cw],
                        add,
                    )
                if po is not None:
                    emit_final_chunk(po, c0, cw)

        pairs = [(i, j) for j in range(3) for i in range(3)]
        cuts = [(c, [CHUNKS[ci]]) for ci, c in enumerate(CUTS)]
        prev = None
        for idx, (i, j) in enumerate(pairs):
            E = e_pool.tile([P, MT, NSLICE], fp8, tag="E")
            if idx == 0:
                emit_q(0)
                emit_k(0, (0,))
            ssplit = SSPLIT
            lo = 0
            for cut_i, (cut, chks) in enumerate(cuts):
                emit_mm1_exp(i, j, E, range(lo, cut), ssplit)
                lo = cut
                extras = {
                    (0, 0): lambda: (emit_k(0, (1,)), emit_q(1), emit_q(2)),
                    (1, 0): lambda: emit_vT_all(),
                    (2, 0): lambda: emit_k(1),
                    (4, 0): lambda: emit_k(2),
                }
                fn = extras.get((idx, cut_i))
                if fn is not None:
                    fn()
                if prev is not None:
                    emit_mm2_norm(prev[0], prev[1], prev[2], chunks=chks)
            emit_mm1_exp(i, j, E, range(lo, MT), ssplit)
            prev = (i, j, E)
        po = [
            big_ps.tile([P, MAIN], f32, tag="big", name="po0"),
            big_ps.tile([P, MAIN], f32, tag="big", name="po1"),
            big_ps.tile([P, MAIN], f32, tag="big", name="poT"),
        ]
        emit_mm2_norm(prev[0], prev[1], prev[2], po=po)

    nc.compile()
    return nc


def _get_program():
    if "nc" not in _CACHE:
        _CACHE["nc"] = _build_program()
    return _CACHE["nc"]


def _host_prep(x, wq, wk, wv, wo):
    import ml_dtypes

    bf16 = ml_dtypes.bfloat16
    fp8 = ml_dtypes.float8_e4m3
    xf = np.ascontiguousarray(x.reshape(B, C, N), dtype=np.float32)
    # wq: [3, R, C] -> wqT: [C, 3, R] -> [P, KT, 3, R] -> dup R to 2R=128.
    # log2e prescale folded into wq (mm1 logits come out as log2e * S).
    wqs = (wq * LOG2E).astype(np.float32)
    def _wt_dup(wmat):
        wT = np.transpose(wmat, (2, 0, 1)).reshape(KT, P, 3, R)  # [KT,P,3,R]
        wT = np.transpose(wT, (1, 0, 2, 3))                      # [P,KT,3,R]
        # zero-pad (NOT duplicate): untiled mm1 contracts all 128
        # partitions, so rows 64:128 of q/k must contribute zero.
        wT = np.concatenate([wT, np.zeros_like(wT)], axis=3)     # [P,KT,3,2R]
        return np.ascontiguousarray(wT).astype(fp8)
    wqT = _wt_dup(wqs)
    wkT = _wt_dup(wk)
    wvT = np.ascontiguousarray(
        np.transpose(np.transpose(wv, (2, 0, 1)).reshape(KT, P, 3, R), (1, 0, 2, 3))
    ).astype(fp8)
    # wo: [C, 3R] -> woT[i] = wo[:, 64i:64(i+1)].T
    woT = np.ascontiguousarray(
        np.stack([wo[:, R * i : R * (i + 1)].T for i in range(3)])
    ).astype(bf16)
    in_maps = []
    for core in range(N_CORES):
        b, h = core // 2, core % 2
        xcore = xf[b].reshape(KT, P, N)
        # rotate columns so this core's n-slice comes first
        xrot = np.concatenate(
            [
                xcore[:, :, h * NSLICE : (h + 1) * NSLICE],
                xcore[:, :, (1 - h) * NSLICE : (2 - h) * NSLICE],
            ],
            axis=2,
        )
        xrot = np.ascontiguousarray(np.transpose(xrot, (1, 0, 2)))  # [P,KT,N]
        in_maps.append(
            {
                "xb": xrot.astype(bf16),
                "x8": xrot.astype(fp8),
                "wqT": wqT,
                "wkT": wkT,
                "wvT": wvT,
                "woT": woT,
            }
        )
    return in_maps


def kernel(x, wq, wk, wv, wo):
    global LAST_RESULTS
    from concourse.bass_utils import run_bass_kernel_spmd

    x = np.asarray(x)
    nc = _get_program()
    in_maps = _host_prep(
        x, np.asarray(wq), np.asarray(wk), np.asarray(wv), np.asarray(wo)
    )
    res = run_bass_kernel_spmd(nc, in_maps, core_ids=list(range(N_CORES)))
    LAST_RESULTS = res
    out = np.empty((B, C, N), np.float32)
    for core in range(N_CORES):
        b, h = core // 2, core % 2
        out[b][:, h * NSLICE : (h + 1) * NSLICE] = (
            res.results[core]["y"].astype(np.float32).reshape(C, NSLICE)
        )
    return out.reshape(B, C, H, W).astype(x.dtype, copy=False)
